# revision 20
# baseline (speedup 1.0000x reference)
"""LoRA Linear kernel for Trainium2 — fp8 DoubleRow main GEMM, fp8 hi/lo LoRA.

Like kernel.py (see its docstring) but the 8.4 MB bf16 x^T copy is replaced
by a 4.2 MB fp8 *residual* xl8 = fp8((x - x8/16)*16), cutting the serial
head from 12.6 MB to 8.4 MB of x traffic. The xa = A x^T prologue runs in
fp8 DoubleRow from (x8 + xl8) against a packed [A_hi | A_lo] stationary
(A_lo the same-scale fp8 residual of A_hi), producing hi/lo product strips
on PSUM partitions 0-15 / 16-31. The ub matmul's K=128 contraction sums the
strips for free: ub rows 0-15 AND 16-31 both hold 2*B^T, row 32 holds
S*bias against the ones-row at xab row 32.

All four xa cross-products share one scale S = 16*1024, which equals the
main-GEMM descale, so no extra rescaling ops are needed anywhere.
"""

import sys
from contextlib import ExitStack

import numpy as np

sys.path.insert(0, "/opt/trn_rl_repo")

import concourse.bacc as bacc  # noqa: E402
import concourse.bass as bass  # noqa: E402
import concourse.mybir as mybir  # noqa: E402
import concourse.tile as tile  # noqa: E402
from concourse.bass import ts  # noqa: E402
from concourse.bass_utils import run_bass_kernel_spmd  # noqa: E402

P = 128
B_DIM, S_DIM = 4, 2048
D = 4096          # in_features (contraction)
O = 4096          # out_features
R = 16            # lora rank
SCALING = 2.0     # alpha / rank = 32/16
NCORES = 8
M = (B_DIM * S_DIM) // NCORES   # tokens per core = 1024
KD = D // P       # 32 contraction tiles of 128
KP = KD // 2      # 16 DoubleRow k-pairs (K=256 each)
MC = 512          # moving free dim per matmul
NMC = M // MC     # 2 m-chunks
NO = O // P       # 32 output-feature tiles

SX = 16.0         # x fp8 pre-scale (xl8 residual uses the same scale)
SW = 1024.0       # W and lora_a fp8 pre-scale (a_lo residual same scale)
S = SX * SW       # main descale; also the intrinsic scale of the xa strips
R2 = 2 * R        # packed [A_hi | A_lo] stationary width
BIAS_ROW = R2     # ones-row index in xab / bias row in ub

FP8 = mybir.dt.float8e4
BF = mybir.dt.bfloat16
F32 = mybir.dt.float32
COPY = mybir.ActivationFunctionType.Copy


def build_program(
    xsplit: int = 4,
    ps_bufs: int = 3,
    wt_bufs: int = 2,
) -> bass.Bass:
    nc = bacc.Bacc()
    xt8 = nc.dram_tensor("xt8", [D, M], FP8, kind="ExternalInput")
    xl8 = nc.dram_tensor("xl8", [D, M], FP8, kind="ExternalInput")
    # W^T pre-packed per output tile: [oi, p, ko, o'] = W^T[ko*128+p, oi*128+o']
    wt8 = nc.dram_tensor("wt8", [NO, P, KD, P], FP8, kind="ExternalInput")
    # A^T hi | lo residual, both *SW: [D, 32]
    at8 = nc.dram_tensor("at8", [D, R2], FP8, kind="ExternalInput")
    # ubb: rows 0..15 = (2*lora_b)^T, rows 16..31 = (2*lora_b)^T (strip sum),
    # row 32 = S*bias, rest 0
    ubb = nc.dram_tensor("ubb", [P, O], BF, kind="ExternalInput")
    # fill for xab rows 32..127: row 32 = ones, rest zeros
    fillb = nc.dram_tensor("fillb", [P - R2, NMC, MC], BF, kind="ExternalInput")
    outT = nc.dram_tensor("outT", [O, M], BF, kind="ExternalOutput")

    xt8_r = xt8.rearrange("(ko p) m -> p ko m", p=P)   # [128, 32, 1024]
    xl8_r = xl8.rearrange("(ko p) m -> p ko m", p=P)   # [128, 32, 1024]
    at8_r = at8.rearrange("(ko p) r -> p ko r", p=P)   # [128, 32, 32]

    with ExitStack() as ctx:
        tc = ctx.enter_context(tile.TileContext(nc))
        xpool = ctx.enter_context(tc.tile_pool(name="xp", bufs=1))
        cpool = ctx.enter_context(tc.tile_pool(name="cpool", bufs=1))
        wt_pool = ctx.enter_context(tc.tile_pool(name="wtp", bufs=wt_bufs))
        out_pool = ctx.enter_context(tc.tile_pool(name="outp", bufs=4))
        ps_pool = ctx.enter_context(
            tc.tile_pool(name="psp", bufs=ps_bufs, space="PSUM")
        )
        psxa_pool = ctx.enter_context(tc.tile_pool(name="psxa", bufs=2, space="PSUM"))

        xt8_sb = xpool.tile([P, KD, M], FP8)
        xl8_sb = xpool.tile([P, KD, M], FP8)
        at_sb = cpool.tile([P, KD, R2], FP8)
        ub_sb = cpool.tile([P, O], BF)            # rows 0..32 real, rest zero
        xab_sb = cpool.tile([P, NMC, MC], BF)     # rows 0..32 real, rest zero

        nc.scalar.dma_start(at_sb[:], at8_r)
        nc.scalar.dma_start(ub_sb[:], ubb[:])
        nc.scalar.dma_start(xab_sb[R2:P, :, :], fillb[:])
        # x8 first (gates the main GEMM and the hi xa chain), residual second.
        XSPLIT = xsplit
        kchunk = KD // XSPLIT
        for h in range(XSPLIT):
            nc.scalar.dma_start(
                xt8_sb[:, ts(h, kchunk), :], xt8_r[:, ts(h, kchunk), :]
            )
        for h in range(XSPLIT):
            nc.scalar.dma_start(
                xl8_sb[:, ts(h, kchunk), :], xl8_r[:, ts(h, kchunk), :]
            )

        # xa strips: [A_hi|A_lo]^T . (x8 + xl8) per m-chunk, DoubleRow fp8.
        # One PSUM accumulation of both chains; strip rows 0-15 (hi) and
        # 16-31 (lo) are summed later by the ub matmul's contraction.
        for mi in range(NMC):
            ps_xa = psxa_pool.tile([R2, MC], F32)
            for ci, xsb in ((0, xt8_sb), (1, xl8_sb)):
                for c in range(KP):
                    nc.tensor.matmul(
                        ps_xa[:],
                        lhsT=at_sb[:, ts(c, 2), :],
                        rhs=xsb[:, ts(c, 2), ts(mi, MC)],
                        start=(ci == 0 and c == 0),
                        stop=(ci == 1 and c == KP - 1),
                        perf_mode=mybir.MatmulPerfMode.DoubleRow,
                    )
            nc.vector.tensor_copy(out=xab_sb[0:R2, mi, :], in_=ps_xa[:])

        # Main: identical to kernel.py
        for oi in range(NO):
            wt_sb = wt_pool.tile([P, KD, P], FP8)
            nc.sync.dma_start(wt_sb[:], wt8[oi])
            ps = [ps_pool.tile([P, MC], F32, name=f"ps{mi}") for mi in range(NMC)]
            for c in range(KP):
                for mi in range(NMC):
                    nc.tensor.matmul(
                        ps[mi][:],
                        lhsT=wt_sb[:, ts(c, 2), :],
                        rhs=xt8_sb[:, ts(c, 2), ts(mi, MC)],
                        start=(c == 0),
                        stop=False,
                        perf_mode=mybir.MatmulPerfMode.DoubleRow,
                    )
            for mi in range(NMC):
                nc.tensor.matmul(
                    ps[mi][:],
                    lhsT=ub_sb[:, ts(oi, P)],
                    rhs=xab_sb[:, mi, :],
                    start=False,
                    stop=True,
                )
                ot = out_pool.tile([P, MC], F32, name="ot")
                nc.scalar.activation(
                    out=ot[:], in_=ps[mi][:], func=COPY, scale=1.0 / S
                )
                nc.gpsimd.dma_start(outT[ts(oi, P), ts(mi, MC)], ot[:])
    nc.compile()
    return nc


def prepare_in_maps(inputs, weight, bias, lora_a, lora_b):
    f8 = mybir.dt.np(FP8)
    bf16 = mybir.dt.np(BF)
    x = np.ascontiguousarray(
        np.asarray(inputs, dtype=np.float32).reshape(B_DIM * S_DIM, D)
    )
    wT = np.asarray(weight, dtype=np.float32).T                      # [D, O]
    wt8 = np.clip(wT * SW, -240.0, 240.0).astype(f8)                 # [D, O]
    wt8_packed = np.ascontiguousarray(
        wt8.reshape(KD, P, NO, P).transpose(2, 1, 0, 3)
    )
    aT = np.asarray(lora_a, dtype=np.float32).T                      # [D, R]
    ah8 = np.clip(aT * SW, -240.0, 240.0).astype(f8)
    al8 = np.clip(
        (aT - ah8.astype(np.float32) / SW) * SW, -240.0, 240.0
    ).astype(f8)
    at8 = np.ascontiguousarray(np.concatenate([ah8, al8], axis=1))   # [D, 32]
    ubb = np.concatenate(
        [
            SCALING * np.asarray(lora_b, dtype=np.float32).T,        # rows 0-15
            SCALING * np.asarray(lora_b, dtype=np.float32).T,        # rows 16-31
            S * np.asarray(bias, dtype=np.float32)[None, :],         # row 32
            np.zeros((P - R2 - 1, O), dtype=np.float32),
        ],
        axis=0,
    ).astype(bf16)
    fillb = np.zeros((P - R2, NMC, MC), dtype=np.float32)
    fillb[0] = 1.0
    fillb = fillb.astype(bf16)
    in_maps = []
    for c in range(NCORES):
        xt_c = np.ascontiguousarray(x[c * M : (c + 1) * M].T)        # [D, M]
        x8 = np.clip(xt_c * SX, -240.0, 240.0).astype(f8)
        xl = np.clip(
            (xt_c - x8.astype(np.float32) / SX) * SX, -240.0, 240.0
        ).astype(f8)
        in_maps.append(
            {
                "xt8": x8,
                "xl8": xl,
                "wt8": wt8_packed,
                "at8": at8,
                "ubb": ubb,
                "fillb": fillb,
            }
        )
    return in_maps


def run(inputs, weight, bias, lora_a, lora_b, trace=False):
    nc = build_program()
    in_maps = prepare_in_maps(inputs, weight, bias, lora_a, lora_b)
    res = run_bass_kernel_spmd(nc, in_maps, list(range(NCORES)), trace=trace)
    shards = [np.asarray(res.results[c]["outT"]).T for c in range(NCORES)]
    out = np.concatenate(shards, axis=0).reshape(B_DIM, S_DIM, O)
    return np.ascontiguousarray(out, dtype=np.float32), res


def kernel(inputs, weight, bias, lora_a, lora_b):
    out, _ = run(inputs, weight, bias, lora_a, lora_b, trace=False)
    return out
